# revision 6
# baseline (speedup 1.0000x reference)
"""Dynamic 3x3 per-pixel filter (DynamicFilterLayer2D) on 8 Trainium2 cores.

Reference: out[b,c,h,w] = sum_{i,j in 3x3} xpad[b,c,h+i,w+j] * f[b,c,(3i+j),h,w]

Sharding: H is split into 8 bands of 32 rows; each core processes all
(b, c) images for its band (data parallel, 1-row halo). Per-core layout:
partitions = 128 (b,c) images (2 groups of 128), free dim = flat pixels.

The kernel streams in fp16 (inputs are quantized host-side; the harness
tolerance is 2e-2 and fp16 keeps the error ~1e-3) which halves the HBM
traffic — the binding resource. Filters are pre-swizzled host-side so
each 4-row super-tile's block is ONE contiguous run per image (tap-major
[tap, row, col] inside the block): the HWDGE emits 128 descriptors per
super instead of 128*9, keeping descriptor generation far ahead of the
SDMA rings. Engine/queue split:

  - DVE: 3 tensor_tensor multiplies per 512-px chunk (one per column tap
    j; the three row taps are batched into one op via an overlapping
    [[W,3],[1,cw]] access pattern on x) -> 9 fp16 product planes
  - PE (tensor): sums the 9 planes via identity-weight matmuls
    accumulating in PSUM (fp32), 512-px chunks (one PSUM bank)
  - ScalarE: one-element-shifted copy of x (the j=1 taps are odd-aligned,
    which would break the DVE 2x mode), the PSUM->SBUF fp32 drain, and
    the output-store HWDGE queue
  - sync (SP) HWDGE queue: filter stream ONLY — nothing else may block it
  - gpsimd SWDGE queue: both groups' x tiles + the tail filter prefetch,
    all issued at kernel start so no mid-stream x load stalls the rings

A 1-element guard at the head of each x row block makes the j=0/j=2 taps
even-aligned; filter border columns (taps that would wrap rows) are
zeroed host-side so no column padding is needed.
"""

import numpy as np

B, C, H, W = 8, 32, 256, 256
K = 3
KK = K * K
N_CORES = 8
BAND = H // N_CORES            # 32 rows per core
RD = 4                         # rows per full super-tile
N_IMG = B * C                  # 256 images
P = 128
N_IMG_GROUPS = N_IMG // P      # 2
FD = RD * W                    # pixels per partition per full super (1024)
CX = 512                       # pixels per DVE product chunk
XS = (RD + 2) * W + 2          # xsh elements per super-tile (guard incl)
X_FLAT = (BAND + 2) * W + 2    # per-image x row storage (guard + pad rows)

# Per-group super-tile schedule (row_start, rows): small lead-in supers so
# the first compute starts after a short DMA; small enders so the final
# compute+store after the last filter bytes arrive is tiny. (30,2) of
# group 1 is prefetched at kernel start so the streamed filter tail ends
# at (28,2).
SUPERS = {
    0: [(0, 2), (2, 2), (4, 4), (8, 4), (12, 4), (16, 4), (20, 4),
        (24, 4), (28, 4)],
    1: [(0, 4), (4, 4), (8, 4), (12, 4), (16, 4), (20, 4), (24, 4),
        (28, 2), (30, 2)],
}
TAIL = (30, 2)                 # prefetched super of group 1

_CACHE = {}


def _strided_ap(tile_ap, dims, offset):
    """Copy of tile_ap with free dims replaced by [[step, count], ...]
    (element units) at element offset; partition dim preserved."""
    import bass_rust
    c = tile_ap.copy()
    part = list(c.ap)[0]
    c.ap = bass_rust.VecI64Pair([list(part)] + [list(d) for d in dims])
    c.offset = offset
    return c


def _build_module():
    import concourse.bacc as bacc
    import concourse.mybir as mybir
    from concourse.tile import TileContext

    fp16 = mybir.dt.float16
    fp32 = mybir.dt.float32
    mult = mybir.AluOpType.mult

    nc = bacc.Bacc("TRN2", target_bir_lowering=False, debug=False)
    x_d = nc.dram_tensor("x_s", [N_IMG, X_FLAT], fp16,
                         kind="ExternalInput").ap()
    f_d = nc.dram_tensor("f_s", [N_IMG, KK * BAND * W], fp16,
                         kind="ExternalInput").ap()
    i_d = nc.dram_tensor("ident", [P, P], fp16, kind="ExternalInput").ap()
    # fp16 output (PSUM accumulates fp32; one rounding on the drain);
    # the host upcasts to fp32 after the gather
    o_d = nc.dram_tensor("o_s", [N_IMG, BAND, W], fp16,
                         kind="ExternalOutput").ap()

    with TileContext(nc) as tc:
        with (
            tc.tile_pool(name="id", bufs=1) as idpool,
            tc.tile_pool(name="xp", bufs=2) as xpool,
            tc.tile_pool(name="xs", bufs=2) as xshpool,
            tc.tile_pool(name="fp", bufs=6) as fpool,
            tc.tile_pool(name="tf", bufs=1) as tailfpool,
            tc.tile_pool(name="pp", bufs=3) as prodpool,
            tc.tile_pool(name="ps", bufs=6, space="PSUM") as psumpool,
            tc.tile_pool(name="op", bufs=2) as opool,
        ):
            ident = idpool.tile([P, P], fp16, tag="id")
            nc.sync.dma_start(out=ident[:, :], in_=i_d[:, :])
            # prefetch both groups' x and the tail filter block up front
            # on the (otherwise idle) SWDGE queue
            xt_g = []
            for g in range(N_IMG_GROUPS):
                xt = xpool.tile([P, X_FLAT], fp16, tag="x")
                nc.gpsimd.dma_start(
                    out=xt[:, :], in_=x_d[g * P:(g + 1) * P, :])
                xt_g.append(xt)
            tr0, trd = TAIL
            tailft = tailfpool.tile([P, KK * trd * W], fp16, tag="tf")
            nc.gpsimd.dma_start(
                out=tailft[:, :],
                in_=f_d[P:2 * P, KK * W * tr0:KK * W * (tr0 + trd)],
            )
            for g in range(N_IMG_GROUPS):
                p0 = g * P
                xt = xt_g[g]
                for (r0, rd) in SUPERS[g]:
                    fd = rd * W
                    xn = fd + 2 * W + 1    # x elems used this super
                    xoff = r0 * W
                    if g == 1 and (r0, rd) == TAIL:
                        ft = tailft
                    else:
                        ft = fpool.tile([P, KK * FD], fp16, tag="f")
                        nc.sync.dma_start(
                            out=ft[:, 0:KK * fd],
                            in_=f_d[p0:p0 + P,
                                    KK * W * r0:KK * W * (r0 + rd)],
                        )
                    # xsh[k] = xt[xoff+k+1]: makes the j=1 taps even-aligned
                    xsh = xshpool.tile([P, XS], fp16, tag="xs")
                    nc.scalar.copy(out=xsh[:, 0:xn - 1],
                                   in_=xt[:, xoff + 1:xoff + xn])

                    # Chunked DVE->PE pipeline (512-px chunks): per-chunk
                    # product tiles let the PE start summing a chunk while
                    # the DVE multiplies the next one.
                    ot = opool.tile([P, FD], fp16, tag="o")
                    c0 = 0
                    while c0 < fd:
                        cw = min(CX, fd - c0)
                        prod = prodpool.tile([P, KK * CX], fp16, tag="pr")
                        # plane t=3i+j: prod[t*cw+p] = x[c0+p+i*W+j] * f_t[c0+p]
                        for j, (src, off) in enumerate(
                                ((xt, xoff), (xsh, 0), (xt, xoff + 2))):
                            in0 = _strided_ap(src[:, :], [[W, K], [1, cw]],
                                              c0 + off)
                            in1 = _strided_ap(ft[:, :], [[K * fd, K], [1, cw]],
                                              j * fd + c0)
                            po = _strided_ap(prod[:, :], [[K * cw, K], [1, cw]],
                                             j * cw)
                            nc.vector.tensor_tensor(po, in0, in1, mult)
                        acc = psumpool.tile([P, CX], fp32, tag="ps")
                        for t in range(KK):
                            nc.tensor.matmul(
                                acc[:, 0:cw],
                                ident[:, :],
                                _strided_ap(prod[:, :], [[1, cw]], t * cw),
                                start=(t == 0),
                                stop=(t == KK - 1),
                            )
                        nc.scalar.copy(out=ot[:, c0:c0 + cw],
                                       in_=acc[:, 0:cw])
                        c0 += cw
                    # outputs ride the Act HWDGE queue so their descriptor
                    # generation never blocks the filter stream (sync queue)
                    nc.scalar.dma_start(
                        out=o_d[p0:p0 + P, r0:r0 + rd, :],
                        in_=ot[:, 0:fd],
                    )
    nc.compile()
    return nc


def _get_module():
    if "nc" not in _CACHE:
        _CACHE["nc"] = _build_module()
    return _CACHE["nc"]


def _shard_inputs(x, dynamic_filters):
    """Per-core input maps. x: [B,C,H,W] f32, filters: [B,C*9,H,W] f32."""
    xp = np.pad(x, ((0, 0), (0, 0), (1, 1), (0, 0))).astype(np.float16)
    # filters -> [B, C, i, j, H, W]; zero the border-column taps (they
    # would multiply out-of-row x elements), then planar tap-major fp16
    f6 = dynamic_filters.reshape(B, C, K, K, H, W).copy()
    f6[:, :, :, 0, :, 0] = 0.0      # j=0 taps multiply x col -1
    f6[:, :, :, 2, :, W - 1] = 0.0  # j=2 taps multiply x col W
    f_pl = f6.reshape(N_IMG, KK, H, W).astype(np.float16)
    ident = np.eye(P, dtype=np.float16)

    in_maps = []
    for n in range(N_CORES):
        r = n * BAND
        xs = xp[:, :, r:r + BAND + 2, :].reshape(N_IMG, (BAND + 2) * W)
        xs_flat = np.zeros((N_IMG, X_FLAT), np.float16)
        xs_flat[:, 1:-1] = xs
        # per-super contiguous blocks ([tap, row, col] within each block)
        # so every filter DMA is one contiguous run per image; block for
        # super (r0, rd) sits at element offset KK*W*r0
        fb = f_pl[:, :, r:r + BAND]          # [N_IMG, KK, BAND, W]
        fs = np.empty((N_IMG, KK * BAND * W), np.float16)
        for g, supers in SUPERS.items():
            imgs = slice(g * P, (g + 1) * P)
            for (r0, rd) in supers:
                fs[imgs, KK * W * r0:KK * W * (r0 + rd)] = (
                    fb[imgs, :, r0:r0 + rd, :].reshape(P, -1))
        in_maps.append({"x_s": xs_flat, "f_s": fs, "ident": ident})
    return in_maps


def kernel(x, dynamic_filters, _trace=False):
    from concourse import bass_utils

    x = np.asarray(x, dtype=np.float32)
    dynamic_filters = np.asarray(dynamic_filters, dtype=np.float32)
    nc = _get_module()
    in_maps = _shard_inputs(x, dynamic_filters)
    res = bass_utils.run_bass_kernel_spmd(
        nc, in_maps, list(range(N_CORES)), trace=_trace)
    out = np.concatenate(
        [res.results[n]["o_s"].reshape(B, C, BAND, W) for n in range(N_CORES)],
        axis=2).astype(np.float32)
    _CACHE["last_exec_time_ns"] = res.exec_time_ns
    return out


# revision 14
# speedup vs baseline: 1.1201x; 1.1201x over previous
"""Dynamic 3x3 per-pixel filter (DynamicFilterLayer2D) on 8 Trainium2 cores.

Reference: out[b,c,h,w] = sum_{i,j in 3x3} xpad[b,c,h+i,w+j] * f[b,c,(3i+j),h,w]

Sharding: H is split into 8 bands of 32 rows; each core processes all
(b, c) images for its band (data parallel, 1-row halo). Per-core layout:
partitions = 128 (b,c) images (2 groups of 128), free dim = flat pixels.

The kernel streams in fp16 (inputs are quantized host-side; the harness
tolerance is 2e-2 and fp16 keeps the error ~1e-3) which halves the HBM
traffic — the binding resource. Filters are pre-swizzled host-side so
each 2-row super-tile's block is ONE contiguous run per image (tap-major
[tap, row, col] inside the block): the HWDGE emits 128 descriptors per
super instead of 128*9, and the fine (2-row / 3.3us) granularity plus a
12-deep tile pool keeps compute tracking the stream with minimal lag.
Engine/queue split:

  - DVE: 3 tensor_tensor multiplies per super (one per column tap j; the
    three row taps are batched into one op via an overlapping
    [[W,3],[1,cw]] access pattern on x) -> 9 fp16 product planes
  - PE (tensor): sums the 9 planes via identity-weight matmuls
    accumulating in PSUM (fp32), one 512-px chunk per super
  - ScalarE: ONE whole-band one-element-shifted copy of x per group at
    kernel start (the j=1 taps are odd-aligned, which would break the
    DVE 2x mode; a single up-front copy keeps it out of the per-super
    dependency chain), the PSUM->SBUF fp32 drain, and the output-store
    HWDGE queue (outputs staged in 8-row blocks for efficient
    descriptors)
  - sync (SP) HWDGE queue: filter stream ONLY — nothing else may block it
  - gpsimd SWDGE queue: both groups' x tiles + the tail filter prefetch,
    all issued at kernel start so no mid-stream x load stalls the rings

A 1-element guard at the head of each x row block makes the j=0/j=2 taps
even-aligned; filter border columns (taps that would wrap rows) are
zeroed host-side so no column padding is needed.
"""

import numpy as np

B, C, H, W = 8, 32, 256, 256
K = 3
KK = K * K
N_CORES = 8
BAND = H // N_CORES            # 32 rows per core
RD = 2                         # rows per super-tile
N_IMG = B * C                  # 256 images
P = 128
N_IMG_GROUPS = N_IMG // P      # 2
FD = RD * W                    # pixels per partition per super (512)
OD = 8                         # rows per output store block
XS = (RD + 2) * W + 2          # xsh elements per super-tile (guard incl)
X_FLAT = (BAND + 2) * W + 2    # per-image x row storage (guard + pad rows)
N_SUP = BAND // RD             # 16 supers per group
TAIL_R0 = BAND - RD            # last super of group 1 is prefetched

_CACHE = {}


def _strided_ap(tile_ap, dims, offset):
    """Copy of tile_ap with free dims replaced by [[step, count], ...]
    (element units) at element offset; partition dim preserved."""
    import bass_rust
    c = tile_ap.copy()
    part = list(c.ap)[0]
    c.ap = bass_rust.VecI64Pair([list(part)] + [list(d) for d in dims])
    c.offset = offset
    return c


def _build_module():
    import concourse.bacc as bacc
    import concourse.mybir as mybir
    from concourse.tile import TileContext

    fp16 = mybir.dt.float16
    fp32 = mybir.dt.float32
    mult = mybir.AluOpType.mult

    nc = bacc.Bacc("TRN2", target_bir_lowering=False, debug=False)
    x_d = nc.dram_tensor("x_s", [N_IMG, X_FLAT], fp16,
                         kind="ExternalInput").ap()
    f_d = nc.dram_tensor("f_s", [N_IMG, KK * BAND * W], fp16,
                         kind="ExternalInput").ap()
    i_d = nc.dram_tensor("ident", [P, P], fp16, kind="ExternalInput").ap()
    # fp16 output (PSUM accumulates fp32; one rounding on the drain);
    # the host upcasts to fp32 after the gather
    o_d = nc.dram_tensor("o_s", [N_IMG, BAND, W], fp16,
                         kind="ExternalOutput").ap()

    with TileContext(nc) as tc:
        with (
            tc.tile_pool(name="id", bufs=1) as idpool,
            tc.tile_pool(name="xp", bufs=2) as xpool,
            tc.tile_pool(name="xs", bufs=2) as xshpool,
            tc.tile_pool(name="fp", bufs=10) as fpool,
            tc.tile_pool(name="tf", bufs=1) as tailfpool,
            tc.tile_pool(name="pp", bufs=3) as prodpool,
            tc.tile_pool(name="ps", bufs=6, space="PSUM") as psumpool,
            tc.tile_pool(name="op", bufs=2) as opool,
        ):
            ident = idpool.tile([P, P], fp16, tag="id")
            nc.sync.dma_start(out=ident[:, :], in_=i_d[:, :])
            # prefetch both groups' x and the tail filter block up front
            # on the (otherwise idle) SWDGE queue
            xt_g, xsh_g = [], []
            for g in range(N_IMG_GROUPS):
                xt = xpool.tile([P, X_FLAT], fp16, tag="x")
                nc.gpsimd.dma_start(
                    out=xt[:, :], in_=x_d[g * P:(g + 1) * P, :])
                xt_g.append(xt)
            tailft = tailfpool.tile([P, KK * FD], fp16, tag="tf")
            nc.gpsimd.dma_start(
                out=tailft[:, :],
                in_=f_d[P:2 * P, KK * W * TAIL_R0:KK * W * (TAIL_R0 + RD)],
            )
            # whole-band shifted copies, queued before any drain so the
            # per-super critical path never touches the scalar queue
            for g in range(N_IMG_GROUPS):
                xsh = xshpool.tile([P, X_FLAT], fp16, tag="xs")
                nc.scalar.copy(out=xsh[:, 0:X_FLAT - 1],
                               in_=xt_g[g][:, 1:X_FLAT])
                xsh_g.append(xsh)
            for g in range(N_IMG_GROUPS):
                p0 = g * P
                xt = xt_g[g]
                ot = None
                for s in range(N_SUP):
                    r0 = s * RD
                    fd = FD
                    cw = fd                # one 512-px chunk per super
                    xoff = r0 * W
                    if g == 1 and r0 == TAIL_R0:
                        ft = tailft
                    else:
                        ft = fpool.tile([P, KK * FD], fp16, tag="f")
                        nc.sync.dma_start(
                            out=ft[:, :],
                            in_=f_d[p0:p0 + P,
                                    KK * W * r0:KK * W * (r0 + RD)],
                        )
                    if ot is None:
                        ot = opool.tile([P, OD * W], fp16, tag="o")
                    oo = (r0 % OD) * W     # offset in the staging tile
                    prod = prodpool.tile([P, KK * FD], fp16, tag="pr")
                    # plane t=3i+j: prod[t*cw+p] = x[p+i*W+j] * f_t[p]
                    for j, (src, off) in enumerate(
                            ((xt, xoff), (xsh_g[g], xoff), (xt, xoff + 2))):
                        in0 = _strided_ap(src[:, :], [[W, K], [1, cw]], off)
                        in1 = _strided_ap(ft[:, :], [[K * fd, K], [1, cw]],
                                          j * fd)
                        po = _strided_ap(prod[:, :], [[K * cw, K], [1, cw]],
                                         j * cw)
                        nc.vector.tensor_tensor(po, in0, in1, mult)
                    acc = psumpool.tile([P, FD], fp32, tag="ps")
                    for t in range(KK):
                        nc.tensor.matmul(
                            acc[:, 0:cw],
                            ident[:, :],
                            _strided_ap(prod[:, :], [[1, cw]], t * cw),
                            start=(t == 0),
                            stop=(t == KK - 1),
                        )
                    nc.scalar.copy(out=ot[:, oo:oo + cw], in_=acc[:, 0:cw])
                    if r0 % OD == OD - RD:
                        # outputs ride the Act HWDGE queue so their
                        # descriptor generation never blocks the filter
                        # stream (sync queue)
                        nc.scalar.dma_start(
                            out=o_d[p0:p0 + P, r0 + RD - OD:r0 + RD, :],
                            in_=ot[:, 0:OD * W],
                        )
                        ot = None
    nc.compile()
    return nc


def _get_module():
    if "nc" not in _CACHE:
        _CACHE["nc"] = _build_module()
    return _CACHE["nc"]


def _shard_inputs(x, dynamic_filters):
    """Per-core input maps. x: [B,C,H,W] f32, filters: [B,C*9,H,W] f32."""
    xp = np.pad(x, ((0, 0), (0, 0), (1, 1), (0, 0))).astype(np.float16)
    # filters -> [B, C, i, j, H, W]; zero the border-column taps (they
    # would multiply out-of-row x elements), then planar tap-major fp16
    f6 = dynamic_filters.reshape(B, C, K, K, H, W).copy()
    f6[:, :, :, 0, :, 0] = 0.0      # j=0 taps multiply x col -1
    f6[:, :, :, 2, :, W - 1] = 0.0  # j=2 taps multiply x col W
    f_pl = f6.reshape(N_IMG, KK, H, W).astype(np.float16)
    ident = np.eye(P, dtype=np.float16)

    in_maps = []
    for n in range(N_CORES):
        r = n * BAND
        xs = xp[:, :, r:r + BAND + 2, :].reshape(N_IMG, (BAND + 2) * W)
        xs_flat = np.zeros((N_IMG, X_FLAT), np.float16)
        xs_flat[:, 1:-1] = xs
        # per-super contiguous blocks ([tap, row, col] within each block)
        # so every filter DMA is one contiguous run per image; block for
        # super r0 sits at element offset KK*W*r0
        fb = f_pl[:, :, r:r + BAND]          # [N_IMG, KK, BAND, W]
        fs = (fb.transpose(0, 2, 1, 3)       # [N_IMG, BAND, KK, W]
                .reshape(N_IMG, N_SUP, RD, KK, W)
                .transpose(0, 1, 3, 2, 4)    # [N_IMG, sup, KK, RD, W]
                .reshape(N_IMG, KK * BAND * W))
        in_maps.append({"x_s": xs_flat, "f_s": fs, "ident": ident})
    return in_maps


def kernel(x, dynamic_filters, _trace=False):
    from concourse import bass_utils

    x = np.asarray(x, dtype=np.float32)
    dynamic_filters = np.asarray(dynamic_filters, dtype=np.float32)
    nc = _get_module()
    in_maps = _shard_inputs(x, dynamic_filters)
    res = bass_utils.run_bass_kernel_spmd(
        nc, in_maps, list(range(N_CORES)), trace=_trace)
    out = np.concatenate(
        [res.results[n]["o_s"].reshape(B, C, BAND, W) for n in range(N_CORES)],
        axis=2).astype(np.float32)
    _CACHE["last_exec_time_ns"] = res.exec_time_ns
    return out
